# revision 26
# baseline (speedup 1.0000x reference)
"""Bass/Tile kernel builder for BSplineField3d (tricubic B-spline interpolation).

Algorithm (per NeuronCore, data-parallel over points):
  Phase 1 (build): from phi [128,128,128,3] build Cy4 in fp16:
      Cy4[x0, yc, z, xs, c, k] = sum_m A[k,m] * phi[x0+xs, yc+m, z, c]
    (x0 in [0,124], xs in [0,4)).  The y-dim B-spline is pre-contracted into
    per-cell polynomial coefficients in v; the 4 x-taps of a point are
    DUPLICATED into every record so that one point needs exactly ONE
    contiguous gather: records of 48 fp16 = [xs4][c3][k4] are contiguous
    along z, so the z-window (4 records = 192 fp16 = 384 B) starting at
    (x0=ix, yc=iy, z=iz) holds everything point-specific.
    Built with fp16 PE matmuls against a banded matrix W[y,(k,yc)], with a
    sliding window of stage tiles (each x-slab feeds 4 stages).
  Phase 2 (points): per chunk of 128x128 points:
    - cell indices + fractional coords on DVE
    - P indirect-DMA gathers (one index per partition per instruction,
      the only vector-mode the HW ucode supports), 384 B per descriptor
    - contraction on DVE in fp16 (packed APs -> 2x perf mode):
        poly-eval in v over k (mult + tree-add), weighted x taps,
        weighted z taps (tree-adds, partially in-place)
"""

from contextlib import ExitStack

import sys as _sys
for _p in ("/opt/trn_rl_repo",):
    if _p not in _sys.path:
        _sys.path.append(_p)

import numpy as np

import concourse.bass as bass
import concourse.tile as tile
from concourse import mybir
from concourse._compat import with_exitstack

F32 = mybir.dt.float32
F16 = mybir.dt.float16
I32 = mybir.dt.int32

NX = 128          # grid points per dim
NCELL = 125       # valid cells per dim (ix in [0,124])
NC_ = 3           # components
ZC = NX * NC_     # 384 floats per (x,y) z-row in phi
REC = 48          # [xs4][c3][k4] fp16 per (x0,yc,z) record in Cy4
ROWE = NX * REC   # 6144 fp16 per (x0,yc)
NRECTOT = NCELL * NCELL * NX   # 2,000,000 records
XSTRIDE = NCELL * NX           # 16000: record-index stride for x0

# variable chunk sizes: small chunks first (gathers can start after only a few
# table slabs are built) and last (short drain tail after the final gather)
CS = [32, 64] + [65] * 28 + [24, 16]
COLS = sum(CS)            # 1956; 128*1956 = 250368 >= 250000
NCHUNK = len(CS)
SMAX = max(CS)            # tile sizing
P = SMAX                  # kept for test.py compatibility

# spacing: dx = 2/(nx-3) = 2/125 -> 1/dx = 62.5; u = (x+1)*62.5
INV_D = 62.5


def bspline_poly_A():
    """A[k][m]: coefficient of v^k in the cubic B-spline weight of tap m."""
    return np.array(
        [
            [1 / 6, 4 / 6, 1 / 6, 0.0],
            [-3 / 6, 0.0, 3 / 6, 0.0],
            [3 / 6, -6 / 6, 3 / 6, 0.0],
            [-1 / 6, 3 / 6, -3 / 6, 1 / 6],
        ],
        dtype=np.float64,
    )


def build_W_const():
    """W[y, k*125+yc] = A[k, y-yc] for 0 <= y-yc <= 3 else 0. Shape [128, 500]."""
    A = bspline_poly_A()
    W = np.zeros((128, 4, 125), np.float32)
    for yc in range(NCELL):
        for m in range(4):
            for k in range(4):
                W[yc + m, k, yc] = A[k, m]
    return W.reshape(128, 500).astype(np.float16)


def _ap(t, offset, dims):
    """Build a raw AP on the same tensor as AP `t` with explicit [step, num] dims."""
    return bass.AP(tensor=t.tensor, offset=t.offset + offset, ap=[list(d) for d in dims])


@with_exitstack
def bspline_kernel(ctx: ExitStack, tc: tile.TileContext, outs, ins, r_by_chunk=None):
    """outs = [T_out [128, COLS, 3] f32]; ins = [xs, ys, zs [128, COLS] f32, phi [128,128,384] f32].

    r_by_chunk: optional per-chunk record-row upper bound (points sorted by ix
    on the host). Bounding each chunk's gather read-range lets the scheduler
    start early chunks' gathers before the full table is built."""
    nc = tc.nc
    xs, ys, zs, phi = ins
    t_out = outs[0]
    if r_by_chunk is None:
        r_by_chunk = [NRECTOT] * NCHUNK

    w_np = build_W_const()
    w_dram = nc.inline_tensor(w_np, name="w_const")

    dram = ctx.enter_context(tc.tile_pool(name="cydram", bufs=1, space="DRAM"))
    cy = dram.tile([NRECTOT, REC], F16)

    add = mybir.AluOpType.add
    sub = mybir.AluOpType.subtract
    mult = mybir.AluOpType.mult
    amin = mybir.AluOpType.min

    # phase-2 prep pools opened early so chunk prep can overlap phase 1
    coords = ctx.enter_context(tc.tile_pool(name="p2_coords", bufs=2))
    small = ctx.enter_context(tc.tile_pool(name="p2_small", bufs=2))
    idxp = ctx.enter_context(tc.tile_pool(name="p2_idx", bufs=4))

    # ---------------- Phase 1: build Cy4 ----------------
    # All pools stay open for the whole program so phase-2 gathers (gated by
    # r_by_chunk read bounds) can overlap the tail of the table build.
    if True:
        singles = ctx.enter_context(tc.tile_pool(name="p1_singles", bufs=1))
        phis = ctx.enter_context(tc.tile_pool(name="p1_phi", bufs=2))
        stages = ctx.enter_context(tc.tile_pool(name="p1_stage", bufs=4))
        psums = ctx.enter_context(tc.psum_pool(name="p1_psum", bufs=2))

        w_sb = singles.tile([128, 500], F16)
        nc.sync.dma_start(out=w_sb[:], in_=w_dram.ap())

        def slot_ap(st, xsl):
            return _ap(st[:], xsl * 12, [[ROWE, NCELL], [REC, NX], [1, 12]])

        stage_by_x0 = {}
        for x in range(NX):
            # load fp32 on SP (keeps Pool free for gathers), cast to fp16 on ACT
            phi_32 = phis.tile([128, ZC], F32, name="phi_in32")
            nc.sync.dma_start(
                out=phi_32[:],
                in_=_ap(phi, x * NX * ZC, [[ZC, 128], [1, ZC]]))
            phi_x = phis.tile([128, ZC], F16, name="phi_in")
            nc.scalar.copy(out=phi_x[:], in_=phi_32[:])
            if True:
                ps = psums.tile([NCELL, 2048], F32)
                for k in range(4):
                    nc.tensor.matmul(
                        ps[:, k * 512:k * 512 + ZC],
                        w_sb[:, k * NCELL:(k + 1) * NCELL],
                        phi_x[:],
                        start=True,
                        stop=True,
                    )
                targets = [(x - xsl, xsl) for xsl in range(4)
                           if 0 <= x - xsl <= NCELL - 1]
                for x0, xsl in targets:
                    if x0 not in stage_by_x0:
                        stage_by_x0[x0] = stages.tile([128, ROWE], F16, name="stage")
                # first target: direct fused fp32 psum -> fp16 stage copy (ACT)
                fx0, fxsl = targets[0]
                fst = stage_by_x0[fx0]
                nc.scalar.copy(
                    out=_ap(fst[:], fxsl * 12,
                            [[ROWE, NCELL], [1, 4], [REC, NX], [4, NC_]]),
                    in_=_ap(ps[:], 0, [[2048, NCELL], [512, 4], [3, NX], [1, NC_]]))
                # remaining targets: fp16 stage->stage copies on DVE (4x mode)
                for x0, xsl in targets[1:]:
                    st = stage_by_x0[x0]
                    nc.vector.tensor_copy(
                        out=slot_ap(st, xsl), in_=slot_ap(fst, fxsl))
                # ship completed stages: stage x0 is complete once x == x0+3
                ship = []
                if x >= 3:
                    ship.append(x - 3)
                if x == NX - 1:
                    ship.extend([NCELL - 3, NCELL - 2, NCELL - 1])
                for x0 in ship:
                    if x0 not in stage_by_x0:
                        continue
                    st = stage_by_x0.pop(x0)
                    # record-rows out AP: per-"row" free = one 48-elem record,
                    # so the cost model's per-partition accounting floors at
                    # descriptor-gen; element sequence identical to the
                    # contiguous [NCELL, ROWE] traversal.
                    nc.sync.dma_start(
                        out=_ap(cy[:], x0 * XSTRIDE * REC,
                                [[REC, NCELL * NX], [1, REC]]),
                        in_=_ap(st[:], 0, [[ROWE, NCELL], [1, ROWE]]),
                    )

    # ---------------- Phase 2: points ----------------
    if True:
        recs = ctx.enter_context(tc.tile_pool(name="p2_rec", bufs=4))
        prods = ctx.enter_context(tc.tile_pool(name="p2_prod", bufs=1))
        touts = ctx.enter_context(tc.tile_pool(name="p2_tout", bufs=2))

        off = 0
        for ch in range(NCHUNK):
            s = CS[ch]
            cy_flat = _ap(cy[:], 0, [[REC, int(r_by_chunk[ch])], [1, REC]])
            # coords layout: [x | z | y] so (u,w) are adjacent for weights
            c3 = coords.tile([128, 3 * s], F32)
            nc.sync.dma_start(out=c3[:, 0:s], in_=xs[:, off:off + s])
            nc.sync.dma_start(out=c3[:, s:2 * s], in_=zs[:, off:off + s])
            nc.sync.dma_start(out=c3[:, 2 * s:3 * s], in_=ys[:, off:off + s])

            # u = (coord+1)*62.5 ; fl = floor(u) (u >= 0) ; fr = u - fl
            nc.vector.tensor_scalar(c3[:], c3[:], 1.0, INV_D, add, mult)
            ci3 = small.tile([128, 3 * s], I32)
            nc.vector.tensor_copy(out=ci3[:], in_=c3[:])
            cf3 = small.tile([128, 3 * s], F32)
            nc.vector.tensor_copy(out=cf3[:], in_=ci3[:])
            fl3 = small.tile([128, 3 * s], F32)
            nc.vector.tensor_tensor(fl3[:], cf3[:], c3[:], mybir.AluOpType.is_gt)
            nc.vector.tensor_tensor(fl3[:], cf3[:], fl3[:], sub)
            frh = small.tile([128, 3 * s], F16)
            nc.vector.tensor_tensor(frh[:], c3[:], fl3[:], sub)
            nc.vector.tensor_scalar(fl3[:], fl3[:], float(NCELL - 1), None, amin)

            # record index: ix*16000 + iy*128 + iz
            idxf = small.tile([128, s], F32)
            nc.vector.scalar_tensor_tensor(
                idxf[:], fl3[:, 2 * s:3 * s], float(NX), fl3[:, s:2 * s], mult, add)
            nc.vector.scalar_tensor_tensor(
                idxf[:], fl3[:, 0:s], float(XSTRIDE), idxf[:], mult, add)
            idxi = idxp.tile([128, s], I32)
            nc.vector.tensor_copy(out=idxi[:], in_=idxf[:])

            # ---- tap weights for u (x) and w (z): wt [128 | g2, s, k4] fp16 ----
            wt = small.tile([128, 2 * s * 4], F16)
            uw = _ap(frh[:], 0, [[3 * s, 128], [1, 2 * s]])

            def wslice(k):
                return _ap(wt[:], k, [[8 * s, 128], [4 * s, 2], [4, s]])

            tg = small.tile([128, 2 * s], F16)
            t2g = small.tile([128, 2 * s], F16)
            r2 = small.tile([128, 2 * s], F16)
            r3 = small.tile([128, 2 * s], F16)
            tmp = small.tile([128, 2 * s], F16)

            def v2(t):  # view [128, 2s] as (2, s)
                return _ap(t[:], 0, [[2 * s, 128], [s, 2], [1, s]])

            nc.vector.tensor_scalar(tg[:], uw, -1.0, 1.0, mult, add)
            nc.vector.tensor_tensor(t2g[:], tg[:], tg[:], mult)
            nc.vector.scalar_tensor_tensor(wslice(0), v2(t2g), 1 / 6, v2(tg), mult, mult)
            nc.vector.tensor_tensor(r2[:], uw, uw, mult)
            nc.vector.tensor_tensor(r3[:], r2[:], uw, mult)
            nc.vector.tensor_scalar(wslice(3), v2(r3), 1 / 6, None, mult)
            nc.vector.scalar_tensor_tensor(tmp[:], r3[:], 0.5, r2[:], mult, sub)
            nc.vector.tensor_scalar(wslice(1), v2(tmp), 2 / 3, None, add)
            nc.vector.tensor_tensor(v2(tmp), wslice(0), wslice(1), add)
            nc.vector.tensor_tensor(v2(tmp), v2(tmp), wslice(3), add)
            nc.vector.tensor_scalar(wslice(2), v2(tmp), -1.0, 1.0, mult, add)

            # ---- v powers: vp4 [128, s, 4] = [1, v, v^2, v^3] fp16 ----
            vp4 = small.tile([128, s * 4], F16)
            frv = _ap(frh[:], 2 * s, [[3 * s, 128], [1, s]])

            def vslot(k):
                return _ap(vp4[:], k, [[4 * s, 128], [4, s]])

            nc.vector.memset(vslot(0), 1.0)
            nc.vector.tensor_copy(out=vslot(1), in_=frv)
            nc.vector.tensor_tensor(vslot(2), frv, frv, mult)
            nc.vector.tensor_tensor(vslot(3), vslot(2), frv, mult)

            # ---- x weights expanded over c: wuc [128, s, x4, c3] fp16 ----
            wuc = small.tile([128, s * 12], F16)
            nc.vector.tensor_copy(
                out=_ap(wuc[:], 0, [[12 * s, 128], [12, s], [3, 4], [1, 3]]),
                in_=_ap(wt[:], 0, [[8 * s, 128], [4, s], [1, 4], [0, 3]]))

            # ---- gather: one record (z-window, 192 fp16) per point ----
            rec = recs.tile([128, s * 192], F16)
            for t in range(s):
                nc.gpsimd.indirect_dma_start(
                    out=_ap(rec[:], t * 192, [[192 * s, 128], [1, 192]]),
                    out_offset=None,
                    in_=cy_flat,
                    in_offset=bass.IndirectOffsetOnAxis(
                        ap=_ap(idxi[:], t, [[s, 128], [1, 1]]), axis=0),
                )

            # ---- contraction ----
            # per point rec = [z4][x4][c3][k4]
            # k poly-eval: rec[pt, zxc48, k4] *= vp4[pt, k4]; tree-add over k
            nc.vector.tensor_tensor(
                _ap(rec[:], 0, [[192 * s, 128], [192, s], [4, 48], [1, 4]]),
                _ap(rec[:], 0, [[192 * s, 128], [192, s], [4, 48], [1, 4]]),
                _ap(vp4[:], 0, [[4 * s, 128], [4, s], [0, 48], [1, 4]]),
                mult)
            nc.vector.tensor_tensor(
                _ap(rec[:], 0, [[192 * s, 128], [192, s], [4, 48], [1, 2]]),
                _ap(rec[:], 0, [[192 * s, 128], [192, s], [4, 48], [1, 2]]),
                _ap(rec[:], 2, [[192 * s, 128], [192, s], [4, 48], [1, 2]]),
                add)
            s1 = prods.tile([128, s * 48], F16)
            nc.vector.tensor_tensor(
                _ap(s1[:], 0, [[48 * s, 128], [48, s], [1, 48]]),
                _ap(rec[:], 0, [[192 * s, 128], [192, s], [4, 48]]),
                _ap(rec[:], 1, [[192 * s, 128], [192, s], [4, 48]]),
                add)
            # x contraction: s1[pt, z4, (x4 c3)12] *= wuc; tree-add over x
            nc.vector.tensor_tensor(
                _ap(s1[:], 0, [[48 * s, 128], [48, s], [12, 4], [1, 12]]),
                _ap(s1[:], 0, [[48 * s, 128], [48, s], [12, 4], [1, 12]]),
                _ap(wuc[:], 0, [[12 * s, 128], [12, s], [0, 4], [1, 12]]),
                mult)
            nc.vector.tensor_tensor(
                _ap(s1[:], 0, [[48 * s, 128], [48, s], [12, 4], [1, 6]]),
                _ap(s1[:], 0, [[48 * s, 128], [48, s], [12, 4], [1, 6]]),
                _ap(s1[:], 6, [[48 * s, 128], [48, s], [12, 4], [1, 6]]),
                add)
            s2 = prods.tile([128, s * 12], F16)
            nc.vector.tensor_tensor(
                _ap(s2[:], 0, [[12 * s, 128], [12, s], [3, 4], [1, 3]]),
                _ap(s1[:], 0, [[48 * s, 128], [48, s], [12, 4], [1, 3]]),
                _ap(s1[:], 3, [[48 * s, 128], [48, s], [12, 4], [1, 3]]),
                add)
            # z contraction: s2[pt, z4, c3] *= ww (bcast over c); tree-add over z
            nc.vector.tensor_tensor(
                _ap(s2[:], 0, [[12 * s, 128], [12, s], [3, 4], [1, 3]]),
                _ap(s2[:], 0, [[12 * s, 128], [12, s], [3, 4], [1, 3]]),
                _ap(wt[:], 4 * s, [[8 * s, 128], [4, s], [1, 4], [0, 3]]),
                mult)
            nc.vector.tensor_tensor(
                _ap(s2[:], 0, [[12 * s, 128], [12, s], [1, 6]]),
                _ap(s2[:], 0, [[12 * s, 128], [12, s], [1, 6]]),
                _ap(s2[:], 6, [[12 * s, 128], [12, s], [1, 6]]),
                add)
            t_c = touts.tile([128, s * 3], F32)
            nc.vector.tensor_tensor(
                _ap(t_c[:], 0, [[3 * s, 128], [3, s], [1, 3]]),
                _ap(s2[:], 0, [[12 * s, 128], [12, s], [1, 3]]),
                _ap(s2[:], 3, [[12 * s, 128], [12, s], [1, 3]]),
                add)

            nc.sync.dma_start(
                out=t_out[:, off:off + s, :],
                in_=t_c[:].rearrange("p (a b) -> p a b", b=3))
            off += s


# ======================================================================
# Self-contained entry point: kernel(**inputs) -> np.ndarray
# ======================================================================

N_POINTS = 2_000_000
N_CORES = 8
PTS_PER_CORE = N_POINTS // N_CORES      # 250000
PAD_PER_CORE = 128 * COLS               # 262144

_CACHE = {}


def _build_nc(r_by_chunk):
    import concourse.bacc as bacc

    nc = bacc.Bacc(
        "TRN2",
        target_bir_lowering=False,
        debug=False,
        num_devices=N_CORES,
    )
    xs = nc.dram_tensor("xs", [128, COLS], F32, kind="ExternalInput").ap()
    ys = nc.dram_tensor("ys", [128, COLS], F32, kind="ExternalInput").ap()
    zs = nc.dram_tensor("zs", [128, COLS], F32, kind="ExternalInput").ap()
    phi = nc.dram_tensor("phi", [128, 128, ZC], F32, kind="ExternalInput").ap()
    t_out = nc.dram_tensor("t_out", [128, COLS, NC_], F32, kind="ExternalOutput").ap()

    with tile.TileContext(nc) as tc:
        bspline_kernel(tc, [t_out], [xs, ys, zs, phi], r_by_chunk=r_by_chunk)
    nc.compile()
    return nc


def get_nc(r_by_chunk=None):
    key = tuple(r_by_chunk) if r_by_chunk is not None else None
    if key not in _CACHE:
        _CACHE[key] = _build_nc(r_by_chunk)
    return _CACHE[key]


_CS_OFF = [0]
for _s in CS:
    _CS_OFF.append(_CS_OFF[-1] + _s)


def _chunk_major(flat):
    """[PAD_PER_CORE] rank-ordered -> [128, COLS]: rank r (within chunk ch of
    size s: r = base + p*s + t) lands at (p, off_ch + t)."""
    A = np.empty((128, COLS), flat.dtype)
    base = 0
    for ch, s in enumerate(CS):
        A[:, _CS_OFF[ch]:_CS_OFF[ch] + s] = flat[base:base + 128 * s].reshape(128, s)
        base += 128 * s
    return np.ascontiguousarray(A)


def _chunk_major_inv(A):
    """[128, COLS, k] -> [PAD_PER_CORE, k] rank-ordered (inverse of _chunk_major)."""
    out = np.empty((PAD_PER_CORE,) + A.shape[2:], A.dtype)
    base = 0
    for ch, s in enumerate(CS):
        out[base:base + 128 * s] = A[:, _CS_OFF[ch]:_CS_OFF[ch] + s].reshape(
            (128 * s,) + A.shape[2:])
        base += 128 * s
    return out


def _sort_shards(x, y, z):
    """Per-core: sort points by ix so each chunk only needs a prefix of the
    table. Returns per-core [128, COLS] coord arrays, per-core perms, and the
    per-chunk record-row bound (max over cores)."""
    xsh, ysh, zsh, perms = [], [], [], []
    xhi = np.zeros((N_CORES, NCHUNK), np.int64)
    for c in range(N_CORES):
        sl = slice(c * PTS_PER_CORE, (c + 1) * PTS_PER_CORE)
        xc = np.asarray(x[sl], np.float32)
        yc = np.asarray(y[sl], np.float32)
        zc = np.asarray(z[sl], np.float32)
        # replicate the device's fp32 cell computation exactly
        u = (xc + np.float32(1.0)) * np.float32(INV_D)
        ix = np.minimum(u.astype(np.int32), NCELL - 1)
        perm = np.argsort(ix, kind="stable")
        perms.append(perm)
        pads = PAD_PER_CORE - PTS_PER_CORE
        xp = np.concatenate([xc[perm], np.zeros(pads, np.float32)])
        yp = np.concatenate([yc[perm], np.zeros(pads, np.float32)])
        zp = np.concatenate([zc[perm], np.zeros(pads, np.float32)])
        ixp = np.concatenate([ix[perm], np.full(pads, 62, np.int32)])
        base = 0
        for ch, s in enumerate(CS):
            xhi[c, ch] = ixp[base:base + 128 * s].max()
            base += 128 * s
        xsh.append(_chunk_major(xp))
        ysh.append(_chunk_major(yp))
        zsh.append(_chunk_major(zp))
    r_by_chunk = ((xhi.max(axis=0) + 1) * XSTRIDE).tolist()
    return xsh, ysh, zsh, perms, r_by_chunk


def run_on_cores(x, y, z, phi_x, trace=False, **kw):
    from concourse.bass_utils import run_bass_kernel_spmd

    xsh, ysh, zsh, perms, r_by_chunk = _sort_shards(x, y, z)
    _CACHE["last_r_by_chunk"] = r_by_chunk
    nc = get_nc(r_by_chunk)
    phi_r = np.ascontiguousarray(phi_x.reshape(128, 128, ZC))
    in_maps = [
        {"xs": xsh[c], "ys": ysh[c], "zs": zsh[c], "phi": phi_r}
        for c in range(N_CORES)
    ]
    res = run_bass_kernel_spmd(
        nc, in_maps, core_ids=list(range(N_CORES)), trace=trace, **kw
    )
    outs = []
    for c in range(N_CORES):
        t = res.results[c]["t_out"]  # [128, COLS, 3], chunk-major sorted order
        t_flat = _chunk_major_inv(t)[:PTS_PER_CORE]
        unsorted = np.empty_like(t_flat)
        unsorted[perms[c]] = t_flat
        outs.append(unsorted)
    full = np.concatenate(outs, axis=0).astype(np.float32)
    return full, res


def kernel(x, y, z, phi_x):
    full, _ = run_on_cores(
        np.asarray(x, dtype=np.float32),
        np.asarray(y, dtype=np.float32),
        np.asarray(z, dtype=np.float32),
        np.asarray(phi_x, dtype=np.float32),
    )
    return full

